# revision 24
# baseline (speedup 1.0000x reference)
"""MoE expert-routing kernel for Trainium2 (8 NeuronCores, expert-parallel).

Problem: out[t] = x[t] @ weight[index[t]] + bias[index[t]]
  x: (32768, 512) f32, index: (32768,) int, weight: (8, 512, 512) f32,
  bias: (8, 512) f32.

Strategy (expert-parallel, host-side dispatch):
  Core e owns expert e. The host gathers the tokens routed to expert e
  into a fixed-capacity, transposed buffer xt_e[512, CAP] (padded with
  zeros), and core e computes y_e = x_e @ W_e + b_e as a single dense
  GEMM. Results are scattered back to token order on the host. Tokens
  beyond CAP (doesn't happen for the benchmark distribution: observed
  per-expert maxima 4205/4166 vs CAP 4224) fall back to a host matmul,
  so the kernel stays correct for any index distribution.

Device kernel (per core): y = x_e @ W_e + b_e over CAP=4224 tokens
  - The host packs x_e pre-transposed AND slab-contiguous: for each
    token-slab, partition p holds one contiguous run of [kc, t] values,
    so every slab DMA is a single 8KB-per-partition contiguous read
    (and the packed fp16 output a 4KB-per-partition write) - no strided
    descriptors anywhere.
  - Token slabs (128/128/256 ramp-in, 512 steady, 256 tail) stream
    through SBUF; per 128-token tile, 4 accumulating matmuls (K=128
    chunks) into one PSUM bank; DVE adds the (pre-replicated) bias while
    moving PSUM->SBUF; outputs go out on the ACT HWDGE ring while inputs
    use the SP ring, so in/out DMAs don't FIFO-block each other.
  - Matmuls use float32r (PE fast-fp32, tf32-like operand precision,
    fp32 PSUM accumulate): sustains ~227ns per [128x128]@[128x512] MM vs
    ~854ns for exact fp32. Output is stored fp16 (halves out-DMA bytes;
    |y| <= ~6 so fp16 rounding adds only ~2^-11 relative error).

Measured (neuron-profile NTFF, per-core exec): ~53-56us, DMA-roofline
bound (14.0MB/core; ~43us DMA busy + ~6us fixed preamble + ramp/tail).
Accuracy vs fp32 reference: absmax 2.3e-3 on scale-5.5 outputs
(4.1e-4 scale-relative); exact-fp32 mode ("float32", ~138us) and
float32r/fp32-out (~68us, 1.4e-4) remain available via KERNEL_MM_DTYPE.
"""

import os

import numpy as np

N_EXPERTS = 8
D_IN = 512
D_OUT = 512
N_TOKENS = 32768
CAP = 4224  # per-expert token capacity: 33*128; observed maxima 4205 (int32 seed) / 4166 (x64); host fallback covers overflow
TOK_SLAB = 512
KC = D_IN // 128  # 4 contraction chunks


def _slab_schedule():
    head_sizes = [128, 128, 256]
    tail_sizes = [256]
    sizes = list(head_sizes)
    remaining = CAP - sum(head_sizes) - sum(tail_sizes)
    while remaining > 0:
        sizes.append(min(TOK_SLAB, remaining))
        remaining -= sizes[-1]
    sizes.extend(tail_sizes)
    slabs = []
    t0 = 0
    for ts in sizes:
        slabs.append((t0, ts))
        t0 += ts
    assert t0 == CAP
    return slabs


SLABS = _slab_schedule()
Y_FREE = (CAP // 128) * D_OUT  # packed output free size per partition

# Measured on HW (exec_time / max-abs-err on scale-5.5 outputs):
#   "float32"      ~138us  5.7e-6   exact fp32 (PE 4 cyc/row)
#   "float32r"      ~68us  7.6e-4   fast-fp32 matmul, fp32 out
#   "float32r_o16"  ~56us  2.3e-3   fast-fp32 matmul, fp16 out  <- default
#   "float16_o16"   ~63us  2.7e-3   fp16 in/out (PE-bound: no 2x stream)
#   "bfloat16"      ~52us  1.3e-2   bf16 in, fp32 out
MM_DTYPE = os.environ.get("KERNEL_MM_DTYPE", "float32r_o16")
# mode -> (x dtype, w dtype, y dtype)
_DT_MAP = {
    "float32": ("float32", "float32", "float32"),
    "float32r": ("float32r", "float32r", "float32"),
    "float32r_o16": ("float32r", "float32r", "float16"),
    "bf16x": ("bfloat16", "float32r", "float32"),
    "bfloat16": ("bfloat16", "bfloat16", "float32"),
    "float16": ("float16", "float16", "float32"),
    "float16_o16": ("float16", "float16", "float16"),
}

_cache = {}


def _build(mm_dtype_name):
    import concourse.bacc as bacc
    import concourse.mybir as mybir
    import concourse.tile as tile

    x_dt_name, w_dt_name, y_dt_name = _DT_MAP[mm_dtype_name]
    dt_x = getattr(mybir.dt, x_dt_name)
    dt_w = getattr(mybir.dt, w_dt_name)
    dt_y = getattr(mybir.dt, y_dt_name)
    f32 = mybir.dt.float32

    nc = bacc.Bacc("TRN2", target_bir_lowering=False, debug=False, num_devices=N_EXPERTS)
    # Slab-contiguous packed layouts: one contiguous run per partition
    # per slab DMA (vs 2KB strided chunks for the natural 2D layouts).
    xt = nc.dram_tensor("xt", (128, KC * CAP), dt_x, kind="ExternalInput").ap()
    w = nc.dram_tensor("w", (D_IN, D_OUT), dt_w, kind="ExternalInput").ap()
    b = nc.dram_tensor("b", (1, D_OUT), f32, kind="ExternalInput").ap()
    y = nc.dram_tensor("y", (128, Y_FREE), dt_y, kind="ExternalOutput").ap()

    with tile.TileContext(nc) as tc:
        with (
            tc.tile_pool(name="wpool", bufs=1) as wpool,
            tc.tile_pool(name="bias", bufs=1) as bias_pool,
            tc.tile_pool(name="xslab", bufs=6) as xpool,
            tc.tile_pool(name="ystage", bufs=6) as ypool,
            tc.tile_pool(name="psum", bufs=6, space="PSUM") as pspool,
            tc.tile_pool(name="psum_b", bufs=1, space="PSUM") as psb_pool,
        ):
            # Slab schedule (module-level, shared with the host packer):
            # small first slabs so matmuls start early, small last slab so
            # the tail flush (DVE + out-DMA after last MM) is short.
            slabs = SLABS

            # Weights: separate tile per k-chunk so the first matmuls only
            # gate on chunk 0 (256KB) instead of the full 1MB.
            w_sbs = [
                wpool.tile([128, D_OUT], dt_w, tag=f"w{k}", name=f"w_sb{k}")
                for k in range(KC)
            ]

            def load_w(k):
                nc.sync.dma_start(w_sbs[k][:], w[k * 128 : (k + 1) * 128, :])

            def load_x(slab_i):
                t0, ts = slabs[slab_i]
                xs = xpool.tile([128, KC * ts], dt_x, tag="xs")
                nc.sync.dma_start(xs[:], xt[:, KC * t0 : KC * (t0 + ts)])
                return xs

            # HWDGE queue order: bias (2KB, feeds the first PE op), w0,
            # x-slab0, w1..w3 — the first GEMM matmul gates on w0+slab0
            # only; remaining W chunks stream behind.
            b_sb1 = bias_pool.tile([1, D_OUT], f32, tag="b1")
            nc.sync.dma_start(b_sb1[:], b[:])
            load_w(0)
            xs_pending = load_x(0)
            for k in range(1, KC):
                load_w(k)

            # Bias: replicate across 128 partitions via ones-matmul
            # (lhsT = ones[1,128], rhs = b[1,512]).
            ones = bias_pool.tile([1, 128], f32, tag="ones")
            nc.any.memset(ones[:], 1.0)
            b_ps = psb_pool.tile([128, D_OUT], f32, tag="bps")
            nc.tensor.matmul(b_ps[:], ones[:], b_sb1[:], start=True, stop=True)
            b_rep = bias_pool.tile([128, D_OUT], f32, tag="brep")
            nc.vector.tensor_copy(b_rep[:], b_ps[:])

            for i, (t0, ts) in enumerate(slabs):
                nt = ts // 128
                xs = xs_pending
                if i + 1 < len(slabs):
                    xs_pending = load_x(i + 1)
                ys = ypool.tile([128, nt * D_OUT], dt_y, tag="ys")
                for a in range(nt):
                    ps = pspool.tile([128, D_OUT], f32, tag="acc")
                    for k in range(KC):
                        nc.tensor.matmul(
                            ps[:],
                            xs[:, k * ts + a * 128 : k * ts + (a + 1) * 128],
                            w_sbs[k][:],
                            start=(k == 0),
                            stop=(k == KC - 1),
                        )
                    nc.vector.tensor_add(
                        ys[:, a * D_OUT : (a + 1) * D_OUT], ps[:], b_rep[:]
                    )
                # Output on the ACT HWDGE ring — separate FIFO from inputs.
                o0 = (t0 // 128) * D_OUT
                nc.scalar.dma_start(y[:, o0 : o0 + nt * D_OUT], ys[:])
    nc.compile()
    return nc


def _get_nc(mm_dtype_name):
    if mm_dtype_name not in _cache:
        _cache[mm_dtype_name] = _build(mm_dtype_name)
    return _cache[mm_dtype_name]


def kernel(x, index, weight, bias, _trace=False):
    from concourse.bass_utils import run_bass_kernel_spmd

    x = np.ascontiguousarray(np.asarray(x, dtype=np.float32))
    weight = np.ascontiguousarray(np.asarray(weight, dtype=np.float32))
    bias = np.ascontiguousarray(np.asarray(bias, dtype=np.float32))
    idx = np.asarray(index).astype(np.int64, copy=False)

    ids = [np.nonzero(idx == e)[0] for e in range(N_EXPERTS)]

    in_maps = []
    for e in range(N_EXPERTS):
        n_e = min(len(ids[e]), CAP)
        x_e = np.zeros((CAP, D_IN), dtype=np.float32)
        x_e[:n_e] = x[ids[e][:n_e]]
        # Pack slab-major: xt_e[p, KC*t0 + kc*ts + t] = x_e[t0+t, kc*128+p]
        xt_e = np.empty((128, KC * CAP), dtype=np.float32)
        for t0, ts in SLABS:
            blk = x_e[t0 : t0 + ts].reshape(ts, KC, 128)  # [t, kc, p]
            xt_e[:, KC * t0 : KC * (t0 + ts)] = (
                blk.transpose(2, 1, 0).reshape(128, KC * ts)
            )
        in_maps.append(
            {
                "xt": xt_e,
                "w": weight[e],
                "b": bias[e : e + 1],
            }
        )

    x_dt_name, w_dt_name, y_dt_name = _DT_MAP[MM_DTYPE]
    _np_dt = {"bfloat16": None, "float16": np.float16}
    if x_dt_name in _np_dt or w_dt_name in _np_dt:
        import ml_dtypes

        cast = {
            "bfloat16": ml_dtypes.bfloat16,
            "float16": np.float16,
        }
        if x_dt_name in cast:
            in_maps = [
                {**m, "xt": m["xt"].astype(cast[x_dt_name])} for m in in_maps
            ]
        if w_dt_name in cast:
            in_maps = [
                {**m, "w": m["w"].astype(cast[w_dt_name])} for m in in_maps
            ]

    nc = _get_nc(MM_DTYPE)
    res = run_bass_kernel_spmd(
        nc, in_maps, core_ids=list(range(N_EXPERTS)), trace=_trace
    )

    out = np.empty((x.shape[0], D_OUT), dtype=np.float32)
    for e in range(N_EXPERTS):
        n_e = min(len(ids[e]), CAP)
        # Unpack [p, a_global, o] -> token-major [a_global*128+p, o]
        y_pm = res.results[e]["y"].reshape(128, CAP // 128, D_OUT)
        y_e = y_pm.transpose(1, 0, 2).reshape(CAP, D_OUT)
        out[ids[e][:n_e]] = y_e[:n_e].astype(np.float32)
        if len(ids[e]) > CAP:  # capacity overflow: host fallback (correctness net)
            over = ids[e][CAP:]
            out[over] = x[over] @ weight[e] + bias[e]

    if _trace:
        return out, res
    return out
